# revision 1
# baseline (speedup 1.0000x reference)
"""MCTC relative-position self-attention on 8 Trainium2 NeuronCores.

Sharding: core = (batch b, head-pair hp): b = core//2, heads {2*hp, 2*hp+1}
of that batch. Each core computes full attention for its 2 heads.

Key trick: rel_pos_rotate(rel)[b,h,i,j] == rel[b,h, M-1+j-i, i], so with
D = q @ E^T of shape [S, L] (L = 2M-1), the rotated matrix is simply
D_flat viewed with row stride L-1 and offset M-1:
    rot[i, j] = D_flat[i*(L-1) + (M-1) + j]
which is a plain strided DMA from a DRAM scratch — no compute.

Matmuls run as float32r (full PE rate at N>=256). Softmax skips the
max-subtraction (scores are O(3), exp is safe in fp32); the 1/sqrt(hd)
scale is folded into the Exp activation's scale; row-sums come from the
activation's accum_out in the same instruction.
"""

import math
import sys

if "/opt/trn_rl_repo" not in sys.path:
    sys.path.insert(0, "/opt/trn_rl_repo")

import numpy as np

import concourse.bass as bass
import concourse.mybir as mybir
import concourse.tile as tile
from concourse import bacc
from concourse.bass_utils import run_bass_kernel_spmd
from concourse.masks import make_identity

S = 920
DMODEL = 1536
HD = 384
M = 920
L = 2 * M - 1  # 1839
NH_PER_CORE = 2

F32 = mybir.dt.float32
# float32r would be 4x faster on the PE but the BIR verifier requires
# producers to pre-round fp32r operands (bitcast alone is rejected).
MM_DT = mybir.dt.float32

P = 128
NS = 8  # ceil(920/128) s-chunks, last has 24 rows
ND = 12  # 1536/128 contraction chunks for projections
NF = 3  # 384/128 feature chunks
NQK = 460  # half of 920, one PSUM bank


def _pc(c):
    return min(P, S - c * P)


def _mm(nc, out, lhsT, rhs, **kw):
    nc.tensor.matmul(out, lhsT.bitcast(MM_DT), rhs.bitcast(MM_DT), **kw)


def build_kernel():
    nc = bacc.Bacc("TRN2", target_bir_lowering=False, debug=False)

    x_d = nc.dram_tensor("x", [S, DMODEL], F32, kind="ExternalInput")
    wq_d = nc.dram_tensor("wq", [DMODEL, NH_PER_CORE * HD], F32, kind="ExternalInput")
    wk_d = nc.dram_tensor("wk", [DMODEL, NH_PER_CORE * HD], F32, kind="ExternalInput")
    wv_d = nc.dram_tensor("wv", [DMODEL, NH_PER_CORE * HD], F32, kind="ExternalInput")
    et_d = nc.dram_tensor("et", [HD, L], F32, kind="ExternalInput")
    out_d = nc.dram_tensor("out", [NH_PER_CORE, S, HD], F32, kind="ExternalOutput")

    from contextlib import ExitStack

    with tile.TileContext(nc) as tc, ExitStack() as ctx:
            ep = ctx.enter_context
            xt_pool = ep(tc.tile_pool(name="xt", bufs=1))
            et_pool = ep(tc.tile_pool(name="et", bufs=1))
            xin_pool = ep(tc.tile_pool(name="xin", bufs=2))
            wch_pool = ep(tc.tile_pool(name="wchunk", bufs=6))
            wv_pool = ep(tc.tile_pool(name="wvres", bufs=1))
            qkt_pool = ep(tc.tile_pool(name="qkt", bufs=1))
            v_pool = ep(tc.tile_pool(name="vsb", bufs=1))
            dst_pool = ep(tc.tile_pool(name="dstage", bufs=3))
            sc_pool = ep(tc.tile_pool(name="sc", bufs=3))
            rel_pool = ep(tc.tile_pool(name="rel", bufs=2))
            pT_pool = ep(tc.tile_pool(name="pT", bufs=1))
            out_pool = ep(tc.tile_pool(name="outp", bufs=2))
            small_pool = ep(tc.tile_pool(name="small", bufs=1))
            pmm = ep(tc.tile_pool(name="pmm", bufs=4, space="PSUM"))
            pv = ep(tc.tile_pool(name="pv", bufs=2, space="PSUM"))
            pt = ep(tc.tile_pool(name="pt", bufs=2, space="PSUM"))
            dram_pool = ep(tc.tile_pool(name="dram", bufs=2, space="DRAM"))

            ident = small_pool.tile([P, P], F32, tag="ident")
            make_identity(nc, ident)

            # ---- load E^T [384, 1839] -> [128, 3, 1839] ----
            et_sb = et_pool.tile([P, NF, L], F32, tag="et")
            et_view = et_d.ap().rearrange("(j p) l -> p j l", p=P)
            for j in range(NF):
                half = L // 2
                nc.sync.dma_start(et_sb[:, j, :half], et_view[:, j, :half])
                nc.sync.dma_start(et_sb[:, j, half:], et_view[:, j, half:])

            # ---- X -> X^T via PE transposes: xt [128, 12, 920] ----
            xt_sb = xt_pool.tile([P, ND, S], F32, tag="xt")
            for c in range(NS):
                pc = _pc(c)
                x_in = xin_pool.tile([P, DMODEL], F32, tag="xin")
                nc.sync.dma_start(
                    x_in[:pc, : DMODEL // 2], x_d.ap()[c * P : c * P + pc, : DMODEL // 2]
                )
                nc.sync.dma_start(
                    x_in[:pc, DMODEL // 2 :], x_d.ap()[c * P : c * P + pc, DMODEL // 2 :]
                )
                for d in range(ND):
                    ps = pt.tile([P, P], F32, tag="pt")
                    nc.tensor.transpose(
                        ps[:P, :pc], x_in[:pc, d * P : (d + 1) * P], ident[:pc, :pc]
                    )
                    nc.vector.tensor_copy(xt_sb[:, d, c * P : c * P + pc], ps[:P, :pc])

            for h in range(NH_PER_CORE):
                hs = h * HD

                # ---- q^T / k^T projections: [384, 920] = W_chunk.T @ X^T ----
                qT_sb = qkt_pool.tile([P, NF, S], F32, tag="qT")
                kT_sb = qkt_pool.tile([P, NF, S], F32, tag="kT")
                for w_d, dst in ((wq_d, qT_sb), (wk_d, kT_sb)):
                    for m in range(NF):
                        ps0 = pmm.tile([P, NQK], F32, tag="pmm")
                        ps1 = pmm.tile([P, NQK], F32, tag="pmm")
                        for kd in range(ND):
                            wch = wch_pool.tile([P, P], F32, tag="wch")
                            nc.sync.dma_start(
                                wch[:],
                                w_d.ap()[
                                    kd * P : (kd + 1) * P, hs + m * P : hs + (m + 1) * P
                                ],
                            )
                            _mm(
                                nc, ps0[:], wch[:], xt_sb[:, kd, :NQK],
                                start=(kd == 0), stop=(kd == ND - 1),
                            )
                            _mm(
                                nc, ps1[:], wch[:], xt_sb[:, kd, NQK:],
                                start=(kd == 0), stop=(kd == ND - 1),
                            )
                        nc.vector.tensor_copy(dst[:, m, :NQK], ps0[:])
                        nc.vector.tensor_copy(dst[:, m, NQK:], ps1[:])

                # ---- v projection (natural layout): [920, 384] ----
                wv_sb = wv_pool.tile([P, ND, HD], F32, tag="wv")
                wv_view = wv_d.ap()[:, hs : hs + HD].rearrange("(j p) f -> p j f", p=P)
                nc.sync.dma_start(wv_sb[:, : ND // 2, :], wv_view[:, : ND // 2, :])
                nc.sync.dma_start(wv_sb[:, ND // 2 :, :], wv_view[:, ND // 2 :, :])
                v_sb = v_pool.tile([P, NS, HD], F32, tag="v")
                for c in range(NS):
                    pc = _pc(c)
                    ps = pv.tile([P, HD], F32, tag="pv")
                    for kd in range(ND):
                        _mm(
                            nc, ps[:pc, :], xt_sb[:, kd, c * P : c * P + pc],
                            wv_sb[:, kd, :],
                            start=(kd == 0), stop=(kd == ND - 1),
                        )
                    nc.vector.tensor_copy(v_sb[:pc, c, :], ps[:pc, :])

                # ---- D = q E^T into DRAM scratch (only needed l-columns) ----
                d_dram = dram_pool.tile([S, L], F32, tag="dscratch")
                d_flat = d_dram.rearrange("a b -> (a b)")
                for c in range(NS):
                    pc = _pc(c)
                    i_max = c * P + pc - 1
                    l_lo = (M - 1) - i_max
                    l_hi = (L - 1) - c * P + 1
                    width = l_hi - l_lo
                    nt = 3
                    base = width // nt
                    sizes = [base + (1 if i < width % nt else 0) for i in range(nt)]
                    off = l_lo
                    for w in sizes:
                        ps = pmm.tile([P, NQK], F32, tag="pmm")
                        for kd in range(NF):
                            _mm(
                                nc, ps[:pc, :w],
                                qT_sb[:, kd, c * P : c * P + pc],
                                et_sb[:, kd, off : off + w],
                                start=(kd == 0), stop=(kd == NF - 1),
                            )
                        dstg = dst_pool.tile([P, NQK], F32, tag="dstg")
                        nc.vector.tensor_copy(dstg[:pc, :w], ps[:pc, :w])
                        nc.sync.dma_start(
                            d_dram[c * P : c * P + pc, off : off + w], dstg[:pc, :w]
                        )
                        off += w

                # ---- scores + rel + exp (+row-sum) per q-chunk ----
                denom = small_pool.tile([P, NS], F32, tag=f"den{h}")
                rden = small_pool.tile([P, NS], F32, tag=f"rden{h}")
                sc_tiles = []
                for c in range(NS):
                    pc = _pc(c)
                    rel_sb = rel_pool.tile([P, S], F32, tag="rel")
                    skew = (
                        d_flat[
                            (M - 1) + c * P * (L - 1) :
                            (M - 1) + c * P * (L - 1) + pc * (L - 1)
                        ]
                        .rearrange("(p x) -> p x", x=L - 1)
                    )
                    nc.sync.dma_start(rel_sb[:pc, :NQK], skew[:, :NQK])
                    nc.sync.dma_start(rel_sb[:pc, NQK:S], skew[:, NQK:S])

                    sc_sb = sc_pool.tile([P, S], F32, tag="sc")
                    for n in range(2):
                        ps = pmm.tile([P, NQK], F32, tag="pmm")
                        for kd in range(NF):
                            _mm(
                                nc, ps[:pc, :],
                                qT_sb[:, kd, c * P : c * P + pc],
                                kT_sb[:, kd, n * NQK : (n + 1) * NQK],
                                start=(kd == 0), stop=(kd == NF - 1),
                            )
                        nc.vector.tensor_add(
                            sc_sb[:pc, n * NQK : (n + 1) * NQK],
                            ps[:pc, :],
                            rel_sb[:pc, n * NQK : (n + 1) * NQK],
                        )
                    nc.scalar.activation(
                        sc_sb[:pc, :],
                        sc_sb[:pc, :],
                        mybir.ActivationFunctionType.Exp,
                        scale=float(1.0 / math.sqrt(HD)),
                        accum_out=denom[:pc, c : c + 1],
                    )
                    nc.vector.reciprocal(rden[:pc, c : c + 1], denom[:pc, c : c + 1])
                    sc_tiles.append(sc_sb)

                # ---- transpose exp(scores) -> probsT [k-part, q] ----
                pT_sb = pT_pool.tile([P, NS, S], F32, tag="pT")
                for c in range(NS):
                    pc = _pc(c)
                    for kc in range(NS):
                        pkc = _pc(kc)
                        ps = pt.tile([P, P], F32, tag="pt")
                        nc.tensor.transpose(
                            ps[:pkc, :pc],
                            sc_tiles[c][:pc, kc * P : kc * P + pkc],
                            ident[:pc, :pc],
                        )
                        nc.vector.tensor_copy(
                            pT_sb[:pkc, kc, c * P : c * P + pc], ps[:pkc, :pc]
                        )

                # ---- ctx = probsT.T @ v, normalized by 1/rowsum ----
                for c in range(NS):
                    pc = _pc(c)
                    ps = pv.tile([P, HD], F32, tag="pv")
                    for kc in range(NS):
                        pkc = _pc(kc)
                        _mm(
                            nc, ps[:pc, :],
                            pT_sb[:pkc, kc, c * P : c * P + pc],
                            v_sb[:pkc, kc, :],
                            start=(kc == 0), stop=(kc == NS - 1),
                        )
                    o_sb = out_pool.tile([P, HD], F32, tag="o")
                    nc.vector.tensor_scalar_mul(
                        o_sb[:pc, :], ps[:pc, :], rden[:pc, c : c + 1]
                    )
                    nc.sync.dma_start(
                        out_d.ap()[h, c * P : c * P + pc, :], o_sb[:pc, :]
                    )

    nc.compile()
    return nc


_NC = None
LAST_RESULTS = None


def kernel(hidden_states, q_w, k_w, v_w, dist_emb):
    global _NC, LAST_RESULTS
    if _NC is None:
        _NC = build_kernel()

    hidden_states = np.asarray(hidden_states, dtype=np.float32)
    q_w = np.asarray(q_w, dtype=np.float32)
    k_w = np.asarray(k_w, dtype=np.float32)
    v_w = np.asarray(v_w, dtype=np.float32)
    dist_emb = np.asarray(dist_emb, dtype=np.float32)

    et = np.ascontiguousarray(dist_emb.T)
    in_maps = []
    for core in range(8):
        b, hp = core // 2, core % 2
        sl = slice(hp * NH_PER_CORE * HD, (hp + 1) * NH_PER_CORE * HD)
        in_maps.append(
            {
                "x": np.ascontiguousarray(hidden_states[b]),
                "wq": np.ascontiguousarray(q_w[:, sl]),
                "wk": np.ascontiguousarray(k_w[:, sl]),
                "wv": np.ascontiguousarray(v_w[:, sl]),
                "et": et,
            }
        )

    res = run_bass_kernel_spmd(_NC, in_maps, core_ids=list(range(8)))
    LAST_RESULTS = res

    B = hidden_states.shape[0]
    out = np.empty((B, S, 4 * HD), np.float32)
    for core in range(8):
        b, hp = core // 2, core % 2
        o = res.results[core]["out"]
        for j in range(NH_PER_CORE):
            h = hp * NH_PER_CORE + j
            out[b, :, h * HD : (h + 1) * HD] = o[j]
    return out



# revision 2
# speedup vs baseline: 2.4389x; 2.4389x over previous
"""MCTC relative-position self-attention on 8 Trainium2 NeuronCores.

Sharding: core = (batch b, head-pair hp): b = core//2, heads {2*hp, 2*hp+1}
of that batch. Each core computes full attention for its 2 heads.

Key trick: rel_pos_rotate(rel)[b,h,i,j] == rel[b,h, M-1+j-i, i], so with
D = q @ E^T of shape [S, L] (L = 2M-1), the rotated matrix is simply
D_flat viewed with row stride L-1 and offset M-1:
    rot[i, j] = D_flat[i*(L-1) + (M-1) + j]
which is a plain strided DMA from a DRAM scratch - no compute.

All matmul operands are fp16: the PE runs 16-bit matmuls at 1 cycle/row
vs 4 for fp32 (accumulation still fp32 in PSUM). fp16's 10-bit mantissa
keeps the overall rel err ~1e-3, well inside the 2e-2 gate; every tensor
here is O(1) so fp16 range is no concern. Inputs are converted to fp16
on the host, halving input DMA traffic too.

Softmax skips the max-subtraction (scores are O(3), exp is safe); the
1/sqrt(hd) scale is folded into the Exp activation's scale; row-sums
come from the activation's accum_out in the same instruction.
"""

import math
import sys

if "/opt/trn_rl_repo" not in sys.path:
    sys.path.insert(0, "/opt/trn_rl_repo")

import numpy as np

import concourse.bass as bass
import concourse.mybir as mybir
import concourse.tile as tile
from concourse import bacc
from concourse.bass_utils import run_bass_kernel_spmd
from concourse.masks import make_identity

S = 920
DMODEL = 1536
HD = 384
M = 920
L = 2 * M - 1  # 1839
NH_PER_CORE = 2

F32 = mybir.dt.float32
F16 = mybir.dt.float16
NP16 = np.float16

P = 128
NS = 8  # ceil(920/128) s-chunks, last has 24 rows
ND = 12  # 1536/128 contraction chunks for projections
NF = 3  # 384/128 feature chunks
NQK = 460  # half of 920, one PSUM bank


def _pc(c):
    return min(P, S - c * P)


def build_kernel():
    nc = bacc.Bacc("TRN2", target_bir_lowering=False, debug=False)

    x_d = nc.dram_tensor("x", [S, DMODEL], F16, kind="ExternalInput")
    wq_d = nc.dram_tensor("wq", [DMODEL, NH_PER_CORE * HD], F16, kind="ExternalInput")
    wk_d = nc.dram_tensor("wk", [DMODEL, NH_PER_CORE * HD], F16, kind="ExternalInput")
    wv_d = nc.dram_tensor("wv", [DMODEL, NH_PER_CORE * HD], F16, kind="ExternalInput")
    et_d = nc.dram_tensor("et", [HD, L], F16, kind="ExternalInput")
    out_d = nc.dram_tensor("out", [NH_PER_CORE, S, HD], F32, kind="ExternalOutput")

    from contextlib import ExitStack

    with tile.TileContext(nc) as tc, ExitStack() as ctx:
            ep = ctx.enter_context
            xt_pool = ep(tc.tile_pool(name="xt", bufs=1))
            et_pool = ep(tc.tile_pool(name="et", bufs=1))
            xin_pool = ep(tc.tile_pool(name="xin", bufs=2))
            wch_pool = ep(tc.tile_pool(name="wchunk", bufs=6))
            wv_pool = ep(tc.tile_pool(name="wvres", bufs=1))
            qkt_pool = ep(tc.tile_pool(name="qkt", bufs=1))
            v_pool = ep(tc.tile_pool(name="vsb", bufs=1))
            dst_pool = ep(tc.tile_pool(name="dstage", bufs=3))
            sc_pool = ep(tc.tile_pool(name="sc", bufs=3))
            rel_pool = ep(tc.tile_pool(name="rel", bufs=2))
            pT_pool = ep(tc.tile_pool(name="pT", bufs=1))
            out_pool = ep(tc.tile_pool(name="outp", bufs=2))
            small_pool = ep(tc.tile_pool(name="small", bufs=1))
            pmm = ep(tc.tile_pool(name="pmm", bufs=4, space="PSUM"))
            pv = ep(tc.tile_pool(name="pv", bufs=2, space="PSUM"))
            pt = ep(tc.tile_pool(name="pt", bufs=2, space="PSUM"))
            dram_pool = ep(tc.tile_pool(name="dram", bufs=2, space="DRAM"))

            ident = small_pool.tile([P, P], F16, tag="ident")
            make_identity(nc, ident)

            # ---- load E^T [384, 1839] -> [128, 3, 1839] ----
            et_sb = et_pool.tile([P, NF, L], F16, tag="et")
            et_view = et_d.ap().rearrange("(j p) l -> p j l", p=P)
            for j in range(NF):
                half = L // 2
                nc.sync.dma_start(et_sb[:, j, :half], et_view[:, j, :half])
                nc.sync.dma_start(et_sb[:, j, half:], et_view[:, j, half:])

            # ---- X -> X^T via PE transposes: xt [128, 12, 920] ----
            xt_sb = xt_pool.tile([P, ND, S], F16, tag="xt")
            for c in range(NS):
                pc = _pc(c)
                x_in = xin_pool.tile([P, DMODEL], F16, tag="xin")
                nc.sync.dma_start(
                    x_in[:pc, : DMODEL // 2], x_d.ap()[c * P : c * P + pc, : DMODEL // 2]
                )
                nc.sync.dma_start(
                    x_in[:pc, DMODEL // 2 :], x_d.ap()[c * P : c * P + pc, DMODEL // 2 :]
                )
                for d in range(ND):
                    ps = pt.tile([P, P], F16, tag="pt")
                    nc.tensor.transpose(
                        ps[:P, :pc], x_in[:pc, d * P : (d + 1) * P], ident[:pc, :pc]
                    )
                    nc.vector.tensor_copy(xt_sb[:, d, c * P : c * P + pc], ps[:P, :pc])

            for h in range(NH_PER_CORE):
                hs = h * HD

                # ---- q^T / k^T projections: [384, 920] = W_chunk.T @ X^T ----
                qT_sb = qkt_pool.tile([P, NF, S], F16, tag="qT")
                kT_sb = qkt_pool.tile([P, NF, S], F16, tag="kT")
                for w_d, dst in ((wq_d, qT_sb), (wk_d, kT_sb)):
                    for m in range(NF):
                        ps0 = pmm.tile([P, NQK], F32, tag="pmm")
                        ps1 = pmm.tile([P, NQK], F32, tag="pmm")
                        for kd in range(ND):
                            wch = wch_pool.tile([P, P], F16, tag="wch")
                            nc.sync.dma_start(
                                wch[:],
                                w_d.ap()[
                                    kd * P : (kd + 1) * P, hs + m * P : hs + (m + 1) * P
                                ],
                            )
                            nc.tensor.matmul(
                                ps0[:], wch[:], xt_sb[:, kd, :NQK],
                                start=(kd == 0), stop=(kd == ND - 1),
                            )
                            nc.tensor.matmul(
                                ps1[:], wch[:], xt_sb[:, kd, NQK:],
                                start=(kd == 0), stop=(kd == ND - 1),
                            )
                        nc.vector.tensor_copy(dst[:, m, :NQK], ps0[:])
                        nc.vector.tensor_copy(dst[:, m, NQK:], ps1[:])

                # ---- v projection (natural layout): [920, 384] ----
                wv_sb = wv_pool.tile([P, ND, HD], F16, tag="wv")
                wv_view = wv_d.ap()[:, hs : hs + HD].rearrange("(j p) f -> p j f", p=P)
                nc.sync.dma_start(wv_sb[:, : ND // 2, :], wv_view[:, : ND // 2, :])
                nc.sync.dma_start(wv_sb[:, ND // 2 :, :], wv_view[:, ND // 2 :, :])
                v_sb = v_pool.tile([P, NS, HD], F16, tag="v")
                for c in range(NS):
                    pc = _pc(c)
                    ps = pv.tile([P, HD], F32, tag="pv")
                    for kd in range(ND):
                        nc.tensor.matmul(
                            ps[:pc, :], xt_sb[:, kd, c * P : c * P + pc],
                            wv_sb[:, kd, :],
                            start=(kd == 0), stop=(kd == ND - 1),
                        )
                    nc.vector.tensor_copy(v_sb[:pc, c, :], ps[:pc, :])

                # ---- D = q E^T into DRAM scratch (only needed l-columns) ----
                d_dram = dram_pool.tile([S, L], F16, tag="dscratch")
                d_flat = d_dram.rearrange("a b -> (a b)")
                for c in range(NS):
                    pc = _pc(c)
                    i_max = c * P + pc - 1
                    l_lo = (M - 1) - i_max
                    l_hi = (L - 1) - c * P + 1
                    width = l_hi - l_lo
                    nt = 3
                    base = width // nt
                    sizes = [base + (1 if i < width % nt else 0) for i in range(nt)]
                    off = l_lo
                    for w in sizes:
                        ps = pmm.tile([P, NQK], F32, tag="pmm")
                        for kd in range(NF):
                            nc.tensor.matmul(
                                ps[:pc, :w],
                                qT_sb[:, kd, c * P : c * P + pc],
                                et_sb[:, kd, off : off + w],
                                start=(kd == 0), stop=(kd == NF - 1),
                            )
                        dstg = dst_pool.tile([P, NQK], F16, tag="dstg")
                        nc.vector.tensor_copy(dstg[:pc, :w], ps[:pc, :w])
                        nc.sync.dma_start(
                            d_dram[c * P : c * P + pc, off : off + w], dstg[:pc, :w]
                        )
                        off += w

                # ---- scores + rel + exp (+row-sum) per q-chunk ----
                denom = small_pool.tile([P, NS], F32, tag=f"den{h}")
                rden = small_pool.tile([P, NS], F32, tag=f"rden{h}")
                sc_tiles = []
                for c in range(NS):
                    pc = _pc(c)
                    rel_sb = rel_pool.tile([P, S], F16, tag="rel")
                    skew = (
                        d_flat[
                            (M - 1) + c * P * (L - 1) :
                            (M - 1) + c * P * (L - 1) + pc * (L - 1)
                        ]
                        .rearrange("(p x) -> p x", x=L - 1)
                    )
                    nc.sync.dma_start(rel_sb[:pc, :NQK], skew[:, :NQK])
                    nc.sync.dma_start(rel_sb[:pc, NQK:S], skew[:, NQK:S])

                    sc_sb = sc_pool.tile([P, S], F16, tag="sc")
                    for n in range(2):
                        ps = pmm.tile([P, NQK], F32, tag="pmm")
                        for kd in range(NF):
                            nc.tensor.matmul(
                                ps[:pc, :],
                                qT_sb[:, kd, c * P : c * P + pc],
                                kT_sb[:, kd, n * NQK : (n + 1) * NQK],
                                start=(kd == 0), stop=(kd == NF - 1),
                            )
                        nc.vector.tensor_add(
                            sc_sb[:pc, n * NQK : (n + 1) * NQK],
                            ps[:pc, :],
                            rel_sb[:pc, n * NQK : (n + 1) * NQK],
                        )
                    nc.scalar.activation(
                        sc_sb[:pc, :],
                        sc_sb[:pc, :],
                        mybir.ActivationFunctionType.Exp,
                        scale=float(1.0 / math.sqrt(HD)),
                        accum_out=denom[:pc, c : c + 1],
                    )
                    nc.vector.reciprocal(rden[:pc, c : c + 1], denom[:pc, c : c + 1])
                    sc_tiles.append(sc_sb)

                # ---- transpose exp(scores) -> probsT [k-part, q] ----
                pT_sb = pT_pool.tile([P, NS, S], F16, tag="pT")
                for c in range(NS):
                    pc = _pc(c)
                    for kc in range(NS):
                        pkc = _pc(kc)
                        ps = pt.tile([P, P], F16, tag="pt")
                        nc.tensor.transpose(
                            ps[:pkc, :pc],
                            sc_tiles[c][:pc, kc * P : kc * P + pkc],
                            ident[:pc, :pc],
                        )
                        nc.vector.tensor_copy(
                            pT_sb[:pkc, kc, c * P : c * P + pc], ps[:pkc, :pc]
                        )

                # ---- ctx = probsT.T @ v, normalized by 1/rowsum ----
                for c in range(NS):
                    pc = _pc(c)
                    ps = pv.tile([P, HD], F32, tag="pv")
                    for kc in range(NS):
                        pkc = _pc(kc)
                        nc.tensor.matmul(
                            ps[:pc, :],
                            pT_sb[:pkc, kc, c * P : c * P + pc],
                            v_sb[:pkc, kc, :],
                            start=(kc == 0), stop=(kc == NS - 1),
                        )
                    o_sb = out_pool.tile([P, HD], F32, tag="o")
                    nc.vector.tensor_scalar_mul(
                        o_sb[:pc, :], ps[:pc, :], rden[:pc, c : c + 1]
                    )
                    nc.sync.dma_start(
                        out_d.ap()[h, c * P : c * P + pc, :], o_sb[:pc, :]
                    )

    nc.compile()
    return nc


_NC = None
LAST_RESULTS = None


def kernel(hidden_states, q_w, k_w, v_w, dist_emb):
    global _NC, LAST_RESULTS
    if _NC is None:
        _NC = build_kernel()

    hidden_states = np.asarray(hidden_states, dtype=np.float32)
    q_w = np.asarray(q_w, dtype=np.float32)
    k_w = np.asarray(k_w, dtype=np.float32)
    v_w = np.asarray(v_w, dtype=np.float32)
    dist_emb = np.asarray(dist_emb, dtype=np.float32)

    et = np.ascontiguousarray(dist_emb.T.astype(NP16))
    in_maps = []
    for core in range(8):
        b, hp = core // 2, core % 2
        sl = slice(hp * NH_PER_CORE * HD, (hp + 1) * NH_PER_CORE * HD)
        in_maps.append(
            {
                "x": np.ascontiguousarray(hidden_states[b].astype(NP16)),
                "wq": np.ascontiguousarray(q_w[:, sl].astype(NP16)),
                "wk": np.ascontiguousarray(k_w[:, sl].astype(NP16)),
                "wv": np.ascontiguousarray(v_w[:, sl].astype(NP16)),
                "et": et,
            }
        )

    res = run_bass_kernel_spmd(_NC, in_maps, core_ids=list(range(8)))
    LAST_RESULTS = res

    B = hidden_states.shape[0]
    out = np.empty((B, S, 4 * HD), np.float32)
    for core in range(8):
        b, hp = core // 2, core % 2
        o = res.results[core]["out"]
        for j in range(NH_PER_CORE):
            h = hp * NH_PER_CORE + j
            out[b, :, h * HD : (h + 1) * HD] = o[j]
    return out


# revision 8
# speedup vs baseline: 2.5205x; 1.0335x over previous
"""MCTC relative-position self-attention on 8 Trainium2 NeuronCores.

Sharding: core = (batch b, head-pair hp): b = core//2, heads {2*hp, 2*hp+1}
of that batch. Each core computes full attention for its 2 heads.

Key trick: rel_pos_rotate(rel)[b,h,i,j] == rel[b,h, M-1+j-i, i], so with
D = q @ E^T of shape [S, L] (L = 2M-1), the rotated matrix is simply
D_flat viewed with row stride L-1 and offset M-1:
    rot[i, j] = D_flat[i*(L-1) + (M-1) + j]
which is a plain strided DMA from a DRAM scratch - no compute.

v2 performance structure:
- All matmul operands fp16: PE runs 1 cycle/row vs 4 for fp32 (PSUM
  accumulation stays fp32). Host converts inputs to fp16.
- Host supplies X^T directly, so no on-chip transpose of X.
- Per-head weights load as one DMA each (few large descriptors).
- D rows stage PSUM -> SBUF as fp16 (casts alternate between the
  vector and scalar engines to balance load), then DMA to DRAM.
- exp(scores) is transposed by the DMA XBAR (dma_start_transpose on
  fp16, one instruction per 128-row q-chunk) instead of PE transposes
  + vector copies.
- scores/exp/transpose/ctx are chunk-local; ctx(c) is emitted two
  chunks behind scores(c) so the PE never waits on the softmax chain.

Softmax skips the max-subtraction (scores are O(3)); the 1/sqrt(hd)
scale is folded into the Exp activation's scale; row-sums come from the
activation's accum_out in the same instruction.
"""

import math
import sys

if "/opt/trn_rl_repo" not in sys.path:
    sys.path.insert(0, "/opt/trn_rl_repo")

import numpy as np

import concourse.bass as bass
import concourse.mybir as mybir
import concourse.tile as tile
from concourse import bacc
from concourse.bass_utils import run_bass_kernel_spmd

S = 920
DMODEL = 1536
HD = 384
M = 920
L = 2 * M - 1  # 1839
NH_PER_CORE = 2

F32 = mybir.dt.float32
F16 = mybir.dt.float16
NP16 = np.float16

P = 128
NS = 8  # ceil(920/128) s-chunks, last has 24 rows
ND = 12  # 1536/128 contraction chunks for projections
NF = 3  # 384/128 feature chunks
NQK = 460  # half of 920, one PSUM bank
SPAD = 1024  # padded score width for the XBAR transpose (8*128)


def _pc(c):
    return min(P, S - c * P)


def build_kernel():
    nc = bacc.Bacc("TRN2", target_bir_lowering=False, debug=False)

    xt_d = nc.dram_tensor("xt", [DMODEL, S], F16, kind="ExternalInput")
    wq_d = nc.dram_tensor("wq", [DMODEL, NH_PER_CORE * HD], F16, kind="ExternalInput")
    wk_d = nc.dram_tensor("wk", [DMODEL, NH_PER_CORE * HD], F16, kind="ExternalInput")
    wv_d = nc.dram_tensor("wv", [DMODEL, NH_PER_CORE * HD], F16, kind="ExternalInput")
    et_d = nc.dram_tensor("et", [HD, L], F16, kind="ExternalInput")
    out_d = nc.dram_tensor("out", [NH_PER_CORE, S, HD], F32, kind="ExternalOutput")

    from contextlib import ExitStack

    with tile.TileContext(nc) as tc, ExitStack() as ctx:
        ep = ctx.enter_context
        xt_pool = ep(tc.tile_pool(name="xt", bufs=1))
        et_pool = ep(tc.tile_pool(name="et", bufs=1))
        w_pool = ep(tc.tile_pool(name="wsb", bufs=1))
        qkt_pool = ep(tc.tile_pool(name="qkt", bufs=1))
        v_pool = ep(tc.tile_pool(name="vsb", bufs=1))
        sc_pool = ep(tc.tile_pool(name="sc", bufs=4))
        dst_pool = ep(tc.tile_pool(name="dstage", bufs=3))
        rel_pool = ep(tc.tile_pool(name="rel", bufs=3))
        pT_pool = ep(tc.tile_pool(name="pT", bufs=1))
        out_pool = ep(tc.tile_pool(name="outp", bufs=2))
        small_pool = ep(tc.tile_pool(name="small", bufs=1))
        pmm = ep(tc.tile_pool(name="pmm", bufs=6, space="PSUM"))
        pv = ep(tc.tile_pool(name="pv", bufs=2, space="PSUM"))
        dram_pool = ep(tc.tile_pool(name="dram", bufs=2, space="DRAM"))

        # ---- load E^T [384, 1839] -> [128, 3, 1839] ----
        et_sb = et_pool.tile([P, NF, L], F16, tag="et")
        et_view = et_d.ap().rearrange("(j p) l -> p j l", p=P)
        for j in range(NF):
            half = L // 2
            nc.sync.dma_start(et_sb[:, j, :half], et_view[:, j, :half])
            nc.sync.dma_start(et_sb[:, j, half:], et_view[:, j, half:])

        # ---- X^T comes from the host: [1536, 920] -> [128, 12, 920] ----
        # One DMA per 128-row d-chunk so the first projection matmuls can
        # start as soon as their chunk lands.
        xt_sb = xt_pool.tile([P, ND, S], F16, tag="xt")
        xt_view = xt_d.ap().rearrange("(j p) s -> p j s", p=P)
        for j in range(ND):
            nc.sync.dma_start(xt_sb[:, j, :], xt_view[:, j, :])

        for h in range(NH_PER_CORE):
            hs = h * HD

            # ---- per-head weights, one DMA each ----
            wq_sb = w_pool.tile([P, ND, HD], F16, tag="wq")
            wk_sb = w_pool.tile([P, ND, HD], F16, tag="wk")
            wv_sb = w_pool.tile([P, ND, HD], F16, tag="wv")
            for w_d, w_sb in ((wq_d, wq_sb), (wk_d, wk_sb), (wv_d, wv_sb)):
                w_view = w_d.ap()[:, hs : hs + HD].rearrange("(j p) f -> p j f", p=P)
                nc.sync.dma_start(w_sb[:, : ND // 2, :], w_view[:, : ND // 2, :])
                nc.sync.dma_start(w_sb[:, ND // 2 :, :], w_view[:, ND // 2 :, :])

            # ---- q^T / k^T projections: [384, 920] = W_chunk.T @ X^T ----
            qT_sb = qkt_pool.tile([P, NF, S], F16, tag="qT")
            kT_sb = qkt_pool.tile([P, NF, S], F16, tag="kT")
            for w_sb, dst in ((wq_sb, qT_sb), (wk_sb, kT_sb)):
                for m in range(NF):
                    ps0 = pmm.tile([P, NQK], F32, tag="pmm")
                    ps1 = pmm.tile([P, NQK], F32, tag="pmm")
                    for kd in range(ND):
                        wch = w_sb[:, kd, m * P : (m + 1) * P]
                        nc.tensor.matmul(
                            ps0[:], wch, xt_sb[:, kd, :NQK],
                            start=(kd == 0), stop=(kd == ND - 1),
                        )
                        nc.tensor.matmul(
                            ps1[:], wch, xt_sb[:, kd, NQK:],
                            start=(kd == 0), stop=(kd == ND - 1),
                        )
                    nc.vector.tensor_copy(dst[:, m, :NQK], ps0[:])
                    nc.vector.tensor_copy(dst[:, m, NQK:], ps1[:])

            # ---- v projection (natural layout): [920, 384] ----
            v_sb = v_pool.tile([P, NS, HD], F16, tag="v")
            for c in range(NS):
                pc = _pc(c)
                ps = pv.tile([P, HD], F32, tag="pv")
                for kd in range(ND):
                    nc.tensor.matmul(
                        ps[:pc, :], xt_sb[:, kd, c * P : c * P + pc],
                        wv_sb[:, kd, :],
                        start=(kd == 0), stop=(kd == ND - 1),
                    )
                nc.vector.tensor_copy(v_sb[:pc, c, :], ps[:pc, :])

            # ---- D = q E^T into DRAM scratch (fp16) ----
            d_dram = dram_pool.tile([S, L], F16, tag="dscratch")
            d_flat = d_dram.rearrange("a b -> (a b)")
            nsplit = 0
            for c in range(NS):
                pc = _pc(c)
                i_max = c * P + pc - 1
                l_lo = (M - 1) - i_max
                l_hi = (L - 1) - c * P + 1
                width = l_hi - l_lo
                nt = 3
                base = width // nt
                sizes = [base + (1 if i < width % nt else 0) for i in range(nt)]
                off = l_lo
                for w in sizes:
                    ps = pmm.tile([P, NQK], F32, tag="pmm")
                    for kd in range(NF):
                        nc.tensor.matmul(
                            ps[:pc, :w],
                            qT_sb[:, kd, c * P : c * P + pc],
                            et_sb[:, kd, off : off + w],
                            start=(kd == 0), stop=(kd == NF - 1),
                        )
                    dstg = dst_pool.tile([P, NQK], F16, tag="dstg")
                    if nsplit % 2 == 0:
                        nc.vector.tensor_copy(dstg[:pc, :w], ps[:pc, :w])
                    else:
                        nc.scalar.copy(dstg[:pc, :w], ps[:pc, :w])
                    nsplit += 1
                    nc.sync.dma_start(
                        d_dram[c * P : c * P + pc, off : off + w], dstg[:pc, :w]
                    )
                    off += w

            # ---- chunk-local pipeline: scores -> exp -> xbar-T -> ctx ----
            denom = small_pool.tile([P, NS], F32, tag=f"den{h}")
            rden = small_pool.tile([P, NS], F32, tag=f"rden{h}")
            pT_sb = pT_pool.tile([P, NS, NS * P], F16, tag="pT")

            def emit_scores(c):
                pc = _pc(c)
                rel_sb = rel_pool.tile([P, S], F16, tag="rel")
                skew = (
                    d_flat[
                        (M - 1) + c * P * (L - 1) :
                        (M - 1) + c * P * (L - 1) + pc * (L - 1)
                    ]
                    .rearrange("(p x) -> p x", x=L - 1)
                )
                nc.sync.dma_start(rel_sb[:pc, :NQK], skew[:, :NQK])
                nc.sync.dma_start(rel_sb[:pc, NQK:S], skew[:, NQK:S])

                sc_sb = sc_pool.tile([P, SPAD], F16, tag="sc")
                # zero the XBAR pad regions (cols S..SPAD, plus the ragged
                # partition rows for the last chunk)
                nc.gpsimd.memset(sc_sb[:, S:], 0.0)
                # ragged last chunk: zero the padded partition rows first
                # (engines can't address a partition base of 24), then the
                # ADD/EXP writes below overwrite the valid rows 0..pc.
                ppad = P if pc == P else ((pc + 15) // 16) * 16
                if ppad > pc:
                    nc.gpsimd.memset(sc_sb[:ppad, :S], 0.0)
                for n in range(2):
                    ps = pmm.tile([P, NQK], F32, tag="pmm")
                    for kd in range(NF):
                        nc.tensor.matmul(
                            ps[:pc, :],
                            qT_sb[:, kd, c * P : c * P + pc],
                            kT_sb[:, kd, n * NQK : (n + 1) * NQK],
                            start=(kd == 0), stop=(kd == NF - 1),
                        )
                    nc.vector.tensor_add(
                        sc_sb[:pc, n * NQK : (n + 1) * NQK],
                        ps[:pc, :],
                        rel_sb[:pc, n * NQK : (n + 1) * NQK],
                    )
                nc.scalar.activation(
                    sc_sb[:pc, :S],
                    sc_sb[:pc, :S],
                    mybir.ActivationFunctionType.Exp,
                    scale=float(1.0 / math.sqrt(HD)),
                    accum_out=denom[:pc, c : c + 1],
                )
                nc.vector.reciprocal(rden[:pc, c : c + 1], denom[:pc, c : c + 1])
                # transpose exp(scores) via the DMA XBAR into pT layout
                nc.sync.dma_start_transpose(
                    pT_sb[:, :, c * P : c * P + ppad], sc_sb[:ppad, :]
                )

            def emit_ctx(c):
                pc = _pc(c)
                ps = pv.tile([P, HD], F32, tag="pv")
                for kc in range(NS):
                    pkc = _pc(kc)
                    nc.tensor.matmul(
                        ps[:pc, :],
                        pT_sb[:pkc, kc, c * P : c * P + pc],
                        v_sb[:pkc, kc, :],
                        start=(kc == 0), stop=(kc == NS - 1),
                    )
                o_sb = out_pool.tile([P, HD], F32, tag="o")
                nc.vector.tensor_scalar_mul(
                    o_sb[:pc, :], ps[:pc, :], rden[:pc, c : c + 1]
                )
                nc.sync.dma_start(out_d.ap()[h, c * P : c * P + pc, :], o_sb[:pc, :])

            LAG = 2
            for c in range(NS + LAG):
                if c < NS:
                    emit_scores(c)
                if c >= LAG:
                    emit_ctx(c - LAG)

    nc.compile()
    return nc


_NC = None
LAST_RESULTS = None


def kernel(hidden_states, q_w, k_w, v_w, dist_emb):
    global _NC, LAST_RESULTS
    if _NC is None:
        _NC = build_kernel()

    hidden_states = np.asarray(hidden_states, dtype=np.float32)
    q_w = np.asarray(q_w, dtype=np.float32)
    k_w = np.asarray(k_w, dtype=np.float32)
    v_w = np.asarray(v_w, dtype=np.float32)
    dist_emb = np.asarray(dist_emb, dtype=np.float32)

    et = np.ascontiguousarray(dist_emb.T.astype(NP16))
    in_maps = []
    for core in range(8):
        b, hp = core // 2, core % 2
        sl = slice(hp * NH_PER_CORE * HD, (hp + 1) * NH_PER_CORE * HD)
        in_maps.append(
            {
                "xt": np.ascontiguousarray(hidden_states[b].T.astype(NP16)),
                "wq": np.ascontiguousarray(q_w[:, sl].astype(NP16)),
                "wk": np.ascontiguousarray(k_w[:, sl].astype(NP16)),
                "wv": np.ascontiguousarray(v_w[:, sl].astype(NP16)),
                "et": et,
            }
        )

    res = run_bass_kernel_spmd(_NC, in_maps, core_ids=list(range(8)))
    LAST_RESULTS = res

    B = hidden_states.shape[0]
    out = np.empty((B, S, 4 * HD), np.float32)
    for core in range(8):
        b, hp = core // 2, core % 2
        o = res.results[core]["out"]
        for j in range(NH_PER_CORE):
            h = hp * NH_PER_CORE + j
            out[b, :, h * HD : (h + 1) * HD] = o[j]
    return out


# revision 11
# speedup vs baseline: 2.8875x; 1.1456x over previous
"""MCTC relative-position self-attention on 8 Trainium2 NeuronCores.

Sharding: core = (batch b, head-pair hp): b = core//2, heads {2*hp, 2*hp+1}
of that batch. Each core computes full attention for its 2 heads.

Key trick: rel_pos_rotate(rel)[b,h,i,j] == rel[b,h, M-1+j-i, i], so with
D = q @ E^T of shape [S, L] (L = 2M-1), the rotated matrix is simply
D_flat viewed with row stride L-1 and offset M-1:
    rot[i, j] = D_flat[i*(L-1) + (M-1) + j]
which is a plain strided DMA from a DRAM scratch - no compute.

v3 performance structure:
- All matmul operands fp16 (PE: 1 cycle/row vs 4 for fp32; PSUM stays
  fp32). Host converts inputs to fp16 and supplies X^T directly.
- Input DMAs ordered by first use (wq, X^T, wk, wv, then E^T, then the
  second head's weights) - DMA here is descriptor-rate-limited
  (~240 GB/s effective), so issue order sets the startup latency.
- D rows stage PSUM -> SBUF fp16 (casts alternate vector/scalar
  engines), then DMA to a DRAM scratch; the rotated matrix comes back
  as a strided skew read.
- exp(scores) is transposed by the DMA XBAR (dma_start_transpose,
  one instruction per q-chunk) instead of PE transposes + copies.
- Head 1's projection matmuls (pure PE, no DMA pressure) are
  interleaved into head 0's attention pipeline so the D/rel/out DMA
  traffic of head 0 hides under them; scores/exp/transpose/ctx stay
  chunk-local with ctx emitted two chunks behind scores.

Softmax skips the max-subtraction (scores are O(3)); the 1/sqrt(hd)
scale is folded into the Exp activation's scale; row-sums come from the
activation's accum_out in the same instruction.
"""

import math
import sys

if "/opt/trn_rl_repo" not in sys.path:
    sys.path.insert(0, "/opt/trn_rl_repo")

import numpy as np

import concourse.bass as bass
import concourse.mybir as mybir
import concourse.tile as tile
from concourse import bacc
from concourse.bass_utils import run_bass_kernel_spmd

S = 920
DMODEL = 1536
HD = 384
M = 920
L = 2 * M - 1  # 1839
NH_PER_CORE = 2

F32 = mybir.dt.float32
F16 = mybir.dt.float16
NP16 = np.float16

P = 128
NS = 8  # ceil(920/128) s-chunks, last has 24 rows
ND = 12  # 1536/128 contraction chunks for projections
NF = 3  # 384/128 feature chunks
NQK = 460  # half of 920, one PSUM bank
SPAD = 1024  # padded score width for the XBAR transpose (8*128)


def _pc(c):
    return min(P, S - c * P)


def build_kernel():
    nc = bacc.Bacc("TRN2", target_bir_lowering=False, debug=False)

    xt_d = nc.dram_tensor("xt", [DMODEL, S], F16, kind="ExternalInput")
    wq_d = nc.dram_tensor("wq", [DMODEL, NH_PER_CORE * HD], F16, kind="ExternalInput")
    wk_d = nc.dram_tensor("wk", [DMODEL, NH_PER_CORE * HD], F16, kind="ExternalInput")
    wv_d = nc.dram_tensor("wv", [DMODEL, NH_PER_CORE * HD], F16, kind="ExternalInput")
    et_d = nc.dram_tensor("et", [HD, L], F16, kind="ExternalInput")
    out_d = nc.dram_tensor("out", [NH_PER_CORE, S, HD], F32, kind="ExternalOutput")

    from contextlib import ExitStack

    with tile.TileContext(nc) as tc, ExitStack() as ctx:
        ep = ctx.enter_context
        xt_pool = ep(tc.tile_pool(name="xt", bufs=1))
        et_pool = ep(tc.tile_pool(name="et", bufs=1))
        w_pool = ep(tc.tile_pool(name="wsb", bufs=1))
        qkt_pool = ep(tc.tile_pool(name="qkt", bufs=2))
        v_pool = ep(tc.tile_pool(name="vsb", bufs=2))
        sc_pool = ep(tc.tile_pool(name="sc", bufs=4))
        dst_pool = ep(tc.tile_pool(name="dstage", bufs=3))
        rel_pool = ep(tc.tile_pool(name="rel", bufs=3))
        pT_pool = ep(tc.tile_pool(name="pT", bufs=1))
        out_pool = ep(tc.tile_pool(name="outp", bufs=2))
        small_pool = ep(tc.tile_pool(name="small", bufs=1))
        pmm = ep(tc.tile_pool(name="pmm", bufs=6, space="PSUM"))
        pv = ep(tc.tile_pool(name="pv", bufs=2, space="PSUM"))
        dram_pool = ep(tc.tile_pool(name="dram", bufs=2, space="DRAM"))

        # ---- input loads, ordered by first use ----
        # wq(h0) first (first matmul), X^T streamed per d-chunk, then
        # wk/wv(h0); E^T (needed at the D phase) and head-1 weights later.
        w_sb = {}
        for h in range(NH_PER_CORE):
            for nm in ("wq", "wk", "wv"):
                w_sb[(nm, h)] = w_pool.tile([P, ND, HD], F16, tag=f"{nm}{h}", name=f"w_{nm}{h}")

        def load_w(nm, w_d, h):
            hs = h * HD
            w_view = w_d.ap()[:, hs : hs + HD].rearrange("(j p) f -> p j f", p=P)
            nc.sync.dma_start(w_sb[(nm, h)][:, : ND // 2, :], w_view[:, : ND // 2, :])
            nc.sync.dma_start(w_sb[(nm, h)][:, ND // 2 :, :], w_view[:, ND // 2 :, :])

        load_w("wq", wq_d, 0)
        xt_sb = xt_pool.tile([P, ND, S], F16, tag="xt")
        xt_view = xt_d.ap().rearrange("(j p) s -> p j s", p=P)
        for j in range(ND):
            nc.sync.dma_start(xt_sb[:, j, :], xt_view[:, j, :])
        load_w("wk", wk_d, 0)
        load_w("wv", wv_d, 0)

        et_sb = et_pool.tile([P, NF, L], F16, tag="et")
        et_view = et_d.ap().rearrange("(j p) l -> p j l", p=P)
        for j in range(NF):
            nc.sync.dma_start(et_sb[:, j, :], et_view[:, j, :])

        load_w("wq", wq_d, 1)
        load_w("wk", wk_d, 1)
        load_w("wv", wv_d, 1)

        # ---- per-head state ----
        qT = {}
        kT = {}
        v_s = {}
        pT = {}
        den = {}
        rde = {}
        dfl = {}

        def proj_slices(h):
            """Return a list of closures, each emitting one PE-contiguous
            slice of head h's q/k/v projections."""
            qT[h] = qkt_pool.tile([P, NF, S], F16, tag="qT", name=f"qT{h}")
            kT[h] = qkt_pool.tile([P, NF, S], F16, tag="kT", name=f"kT{h}")
            v_s[h] = v_pool.tile([P, NS, HD], F16, tag="v", name=f"v{h}")
            slices = []

            def qk_slice(w, dst, m):
                def emit():
                    ps0 = pmm.tile([P, NQK], F32, tag="pmm")
                    ps1 = pmm.tile([P, NQK], F32, tag="pmm")
                    for kd in range(ND):
                        wch = w[:, kd, m * P : (m + 1) * P]
                        nc.tensor.matmul(
                            ps0[:], wch, xt_sb[:, kd, :NQK],
                            start=(kd == 0), stop=(kd == ND - 1),
                        )
                        nc.tensor.matmul(
                            ps1[:], wch, xt_sb[:, kd, NQK:],
                            start=(kd == 0), stop=(kd == ND - 1),
                        )
                    nc.vector.tensor_copy(dst[:, m, :NQK], ps0[:])
                    nc.vector.tensor_copy(dst[:, m, NQK:], ps1[:])
                return emit

            def v_slice(c0, c1):
                def emit():
                    for c in range(c0, c1):
                        pc = _pc(c)
                        ps = pv.tile([P, HD], F32, tag="pv")
                        for kd in range(ND):
                            nc.tensor.matmul(
                                ps[:pc, :], xt_sb[:, kd, c * P : c * P + pc],
                                w_sb[("wv", h)][:, kd, :],
                                start=(kd == 0), stop=(kd == ND - 1),
                            )
                        nc.vector.tensor_copy(v_s[h][:pc, c, :], ps[:pc, :])
                return emit

            for m in range(NF):
                slices.append(qk_slice(w_sb[("wq", h)], qT[h], m))
            for m in range(NF):
                slices.append(qk_slice(w_sb[("wk", h)], kT[h], m))
            for c0 in range(0, NS, 2):
                slices.append(v_slice(c0, min(c0 + 2, NS)))
            return slices

        def emit_D(h):
            d_dram = dram_pool.tile([S, L], F16, tag="dscratch")
            dfl[h] = d_dram.rearrange("a b -> (a b)")
            nsplit = 0
            for c in range(NS):
                pc = _pc(c)
                i_max = c * P + pc - 1
                l_lo = (M - 1) - i_max
                l_hi = (L - 1) - c * P + 1
                width = l_hi - l_lo
                nt = 3
                base = width // nt
                sizes = [base + (1 if i < width % nt else 0) for i in range(nt)]
                off = l_lo
                for w in sizes:
                    ps = pmm.tile([P, NQK], F32, tag="pmm")
                    for kd in range(NF):
                        nc.tensor.matmul(
                            ps[:pc, :w],
                            qT[h][:, kd, c * P : c * P + pc],
                            et_sb[:, kd, off : off + w],
                            start=(kd == 0), stop=(kd == NF - 1),
                        )
                    dstg = dst_pool.tile([P, NQK], F16, tag="dstg")
                    if nsplit % 2 == 0:
                        nc.vector.tensor_copy(dstg[:pc, :w], ps[:pc, :w])
                    else:
                        nc.scalar.copy(dstg[:pc, :w], ps[:pc, :w])
                    nsplit += 1
                    nc.sync.dma_start(
                        d_dram[c * P : c * P + pc, off : off + w], dstg[:pc, :w]
                    )
                    off += w

        def emit_scores(h, c):
            pc = _pc(c)
            rel_sb = rel_pool.tile([P, S], F16, tag="rel")
            skew = (
                dfl[h][
                    (M - 1) + c * P * (L - 1) :
                    (M - 1) + c * P * (L - 1) + pc * (L - 1)
                ]
                .rearrange("(p x) -> p x", x=L - 1)
            )
            nc.sync.dma_start(rel_sb[:pc, :NQK], skew[:, :NQK])
            nc.sync.dma_start(rel_sb[:pc, NQK:S], skew[:, NQK:S])

            sc_sb = sc_pool.tile([P, SPAD], F16, tag="sc")
            nc.gpsimd.memset(sc_sb[:, S:], 0.0)
            # ragged last chunk: zero the padded partition rows first
            # (engines can't address a partition base of 24), then the
            # ADD/EXP writes below overwrite the valid rows 0..pc.
            ppad = P if pc == P else ((pc + 15) // 16) * 16
            if ppad > pc:
                nc.gpsimd.memset(sc_sb[:ppad, :S], 0.0)
            for n in range(2):
                ps = pmm.tile([P, NQK], F32, tag="pmm")
                for kd in range(NF):
                    nc.tensor.matmul(
                        ps[:pc, :],
                        qT[h][:, kd, c * P : c * P + pc],
                        kT[h][:, kd, n * NQK : (n + 1) * NQK],
                        start=(kd == 0), stop=(kd == NF - 1),
                    )
                nc.vector.tensor_add(
                    sc_sb[:pc, n * NQK : (n + 1) * NQK],
                    ps[:pc, :],
                    rel_sb[:pc, n * NQK : (n + 1) * NQK],
                )
            nc.scalar.activation(
                sc_sb[:pc, :S],
                sc_sb[:pc, :S],
                mybir.ActivationFunctionType.Exp,
                scale=float(1.0 / math.sqrt(HD)),
                accum_out=den[h][:pc, c : c + 1],
            )
            nc.vector.reciprocal(rde[h][:pc, c : c + 1], den[h][:pc, c : c + 1])
            nc.sync.dma_start_transpose(
                pT[h][:, :, c * P : c * P + ppad], sc_sb[:ppad, :]
            )

        def emit_ctx(h, c):
            pc = _pc(c)
            ps = pv.tile([P, HD], F32, tag="pv")
            for kc in range(NS):
                pkc = _pc(kc)
                nc.tensor.matmul(
                    ps[:pc, :],
                    pT[h][:pkc, kc, c * P : c * P + pc],
                    v_s[h][:pkc, kc, :],
                    start=(kc == 0), stop=(kc == NS - 1),
                )
            o_sb = out_pool.tile([P, HD], F32, tag="o")
            nc.vector.tensor_scalar_mul(
                o_sb[:pc, :], ps[:pc, :], rde[h][:pc, c : c + 1]
            )
            nc.sync.dma_start(out_d.ap()[h, c * P : c * P + pc, :], o_sb[:pc, :])

        def emit_attention(h, filler):
            """Chunk-local pipeline; `filler` is a list of closures (other
            PE work) drained between pipeline stages to hide DMA latency."""
            den[h] = small_pool.tile([P, NS], F32, tag=f"den{h}", name=f"den{h}")
            rde[h] = small_pool.tile([P, NS], F32, tag=f"rden{h}", name=f"rden{h}")
            pT[h] = pT_pool.tile([P, NS, NS * P], F16, tag="pT", name=f"pT{h}")
            LAG = 2
            for c in range(NS + LAG):
                if c < NS:
                    emit_scores(h, c)
                if filler:
                    filler.pop(0)()
                if filler and c % 2 == 1:
                    filler.pop(0)()
                if c >= LAG:
                    emit_ctx(h, c - LAG)
            for f in filler:
                f()
            filler.clear()

        # head 0: projections, D, then attention with head 1's projection
        # slices as PE filler between the DMA-bound pipeline stages.
        for f in proj_slices(0):
            f()
        emit_D(0)
        h1_slices = proj_slices(1)
        emit_attention(0, h1_slices)
        emit_D(1)
        emit_attention(1, [])

    nc.compile()
    return nc


_NC = None
LAST_RESULTS = None


def kernel(hidden_states, q_w, k_w, v_w, dist_emb):
    global _NC, LAST_RESULTS
    if _NC is None:
        _NC = build_kernel()

    hidden_states = np.asarray(hidden_states, dtype=np.float32)
    q_w = np.asarray(q_w, dtype=np.float32)
    k_w = np.asarray(k_w, dtype=np.float32)
    v_w = np.asarray(v_w, dtype=np.float32)
    dist_emb = np.asarray(dist_emb, dtype=np.float32)

    et = np.ascontiguousarray(dist_emb.T.astype(NP16))
    in_maps = []
    for core in range(8):
        b, hp = core // 2, core % 2
        sl = slice(hp * NH_PER_CORE * HD, (hp + 1) * NH_PER_CORE * HD)
        in_maps.append(
            {
                "xt": np.ascontiguousarray(hidden_states[b].T.astype(NP16)),
                "wq": np.ascontiguousarray(q_w[:, sl].astype(NP16)),
                "wk": np.ascontiguousarray(k_w[:, sl].astype(NP16)),
                "wv": np.ascontiguousarray(v_w[:, sl].astype(NP16)),
                "et": et,
            }
        )

    res = run_bass_kernel_spmd(_NC, in_maps, core_ids=list(range(8)))
    LAST_RESULTS = res

    B = hidden_states.shape[0]
    out = np.empty((B, S, 4 * HD), np.float32)
    for core in range(8):
        b, hp = core // 2, core % 2
        o = res.results[core]["out"]
        for j in range(NH_PER_CORE):
            h = hp * NH_PER_CORE + j
            out[b, :, h * HD : (h + 1) * HD] = o[j]
    return out
